# revision 11
# baseline (speedup 1.0000x reference)
"""Trainium2 Bass kernel for nn_CustomGINConv (gnn_message_passing).

Reference computation (per path n, L=6 layers, C=128 channels):
    h[l]    = x[l] @ Wt[:C] + emb[idx[l]] @ Wt[C:] + bt
    prop[l] = h[l-1] + h[l+1]                (zero-padded along l)
    u[l]    = (1+eps) * x[l] + prop[l]
    out     = sum_l relu(u[l] @ W1 + b1) @ W2 + b2   -> [N, C]

Kernel strategy (shard N across 8 cores, feature-major on-chip layout):
  * eps is folded into weights host-side:
      Wtx' = Wt[:C]/(1+eps),  T' = (emb @ Wt[C:] + bt)/(1+eps),  W1' = (1+eps)*W1
    so   u[l]/(1+eps) = x[l] + x[l-1]@Wtx' + x[l+1]@Wtx' + T'[idx[l-1]] + T'[idx[l+1]]
    and  relu(u@W1 + b1) = relu((u/(1+eps)) @ W1' + b1).
  * The shift-add along l is linear, so it is absorbed into the PSUM
    accumulation of the tensor engine (no vector shift-adds at all).
  * The embedding gather is a one-hot matmul: the host precomputes
    ohsum[l] = onehot(idx[l-1]) + onehot(idx[l+1]) in fp8e4m3 (exact 0/1/2)
    and the PE contracts it against T' (fp8) into the same PSUM banks.
  * fp32 matmuls use the float32r dtype (full-rate fp32 on TRN2 when the
    moving dim is >= 256).
  * Per 512-column tile: 16 prop matmuls -> 1 stacked DVE add (u, in place)
    -> per l: W1' matmul -> ACT relu+bias -> W2 matmul accumulated into one
    PSUM bank -> ACT copy + 6*b2 bias -> DMA out.
"""

import os
import sys

import numpy as np

sys.path.insert(0, "/opt/trn_rl_repo")

import ml_dtypes  # noqa: E402

import concourse.bass as bass  # noqa: E402
import concourse.tile as tile  # noqa: E402
from concourse import bacc, mybir  # noqa: E402
from concourse import bass_utils  # noqa: E402
from concourse.bass import ts  # noqa: E402

L = 6
N_FULL = 65536
C = 128
EMB = 100
NCORES = 8
NC_N = N_FULL // NCORES  # 8192 rows per core
M = 512  # tile width (columns of the feature-major layout)

F32 = mybir.dt.float32
F32R = mybir.dt.float32r
BF16 = mybir.dt.bfloat16
F8 = mybir.dt.float8e4

# x-path precision: float32r (full fp32 accuracy) or bfloat16 (halves the
# dominant HBM payload; ~0.2% relative output error). Toggle before build_bass().
X_BF16 = os.environ.get("KERNEL_X_BF16", "1") == "1"
# which layers run their relu on DVE instead of ACT (load balancing)
RELU_DVE_LS = (1, 3, 5)
RELU = mybir.ActivationFunctionType.Relu
IDENT = mybir.ActivationFunctionType.Identity

# fp8e4m3 bit patterns for {0, 1/16, 2/16}: the one-hot carries a 1/16
# factor (exact powers of two) and tw1 is pre-scaled by 16 so its small
# entries sit in fp8's normal range instead of the subnormals.
_FP8_LUT = np.array([0x00, 0x18, 0x20], dtype=np.uint8)
_OH_SCALE = np.float32(16.0)


def build_bass(nc_n: int = NC_N, num_devices: int = NCORES) -> bass.Bass:
    """Build + compile the per-core Bass program (same program on all cores)."""
    nc = bacc.Bacc(
        "TRN2",
        target_bir_lowering=False,
        debug=False,
        enable_asserts=False,
        num_devices=num_devices,
    )
    XDT = BF16 if X_BF16 else F32R
    xt = nc.dram_tensor("xt", [C, L, nc_n], XDT, kind="ExternalInput").ap()
    oh = nc.dram_tensor("oh", [EMB, L, nc_n], F8, kind="ExternalInput").ap()
    w1d = nc.dram_tensor("w1d", [C, C], XDT, kind="ExternalInput").ap()
    w1x = nc.dram_tensor("w1x", [C, C], XDT, kind="ExternalInput").ap()
    tw1 = nc.dram_tensor("tw1", [EMB, C], F8, kind="ExternalInput").ap()
    w2 = nc.dram_tensor("w2", [C, C], F32R, kind="ExternalInput").ap()
    b1 = nc.dram_tensor("b1", [C, 1], F32, kind="ExternalInput").ap()
    b2s = nc.dram_tensor("b2s", [C, 1], F32, kind="ExternalInput").ap()
    out = nc.dram_tensor("out", [C, nc_n], F32, kind="ExternalOutput").ap()

    nt = nc_n // M
    with tile.TileContext(nc) as tc:
        with (
            tc.tile_pool(name="consts", bufs=1) as consts,
            tc.tile_pool(name="xp", bufs=2) as xp,
            tc.tile_pool(name="ohp", bufs=2) as ohp,
            tc.tile_pool(name="zp", bufs=4) as zp,
            tc.tile_pool(name="outp", bufs=2) as outp,
            tc.tile_pool(name="pp", bufs=1, space="PSUM") as pp,
        ):
            w1d_sb = consts.tile([C, C], XDT, tag="w1d")
            nc.sync.dma_start(w1d_sb[:], w1d)
            w1x_sb = consts.tile([C, C], XDT, tag="w1x")
            nc.sync.dma_start(w1x_sb[:], w1x)
            tw1_sb = consts.tile([EMB, C], F8, tag="tw1")
            nc.sync.dma_start(tw1_sb[:], tw1)
            w2_sb = consts.tile([C, C], F32R, tag="w2")
            nc.sync.dma_start(w2_sb[:], w2)
            b1_sb = consts.tile([C, 1], F32, tag="b1")
            nc.sync.dma_start(b1_sb[:], b1)
            b2_sb = consts.tile([C, 1], F32, tag="b2")
            nc.sync.dma_start(b2_sb[:], b2s)

            for i in range(nt):
                xt_t = xp.tile([C, L, M], XDT, tag="xt")
                nc.sync.dma_start(xt_t[:], xt[:, :, ts(i, M)])
                oh_t = ohp.tile([EMB, L, M], F8, tag="oh")
                nc.sync.dma_start(oh_t[:], oh[:, :, ts(i, M)])

                # xs[l] = x[l-1] + x[l+1] for interior l (one stacked DVE op);
                # boundary layers use the single neighbor directly.
                xs_t = xp.tile([C, L - 2, M], XDT, tag="xs")
                nc.vector.tensor_tensor(
                    xs_t[:], xt_t[:, 0 : L - 2, :], xt_t[:, 2:L, :],
                    mybir.AluOpType.add,
                )

                # z1[l] (pre-relu) accumulates directly in a PSUM bank:
                #   W1'^T x[l] + (Wtx@W1)^T (x[l-1]+x[l+1]) + (T@W1)^T ohsum[l]
                y_ps = pp.tile([C, M], F32, tag="y", bufs=2)
                out_t = outp.tile([C, M], F32, tag="out")
                for l in range(L):
                    z_ps = pp.tile([C, M], F32, tag="z1", bufs=6)
                    nc.tensor.matmul(
                        z_ps[:], w1d_sb[:], xt_t[:, l, :], start=True, stop=False
                    )
                    nbr = (
                        xt_t[:, 1, :] if l == 0
                        else xt_t[:, L - 2, :] if l == L - 1
                        else xs_t[:, l - 1, :]
                    )
                    nc.tensor.matmul(z_ps[:], w1x_sb[:], nbr, start=False, stop=False)
                    nc.tensor.matmul(
                        z_ps[:], tw1_sb[:], oh_t[:, l, :], start=False, stop=True
                    )
                    z_sb = zp.tile([C, M], F32R, tag="z")
                    if l in RELU_DVE_LS:
                        nc.vector.tensor_scalar(
                            z_sb[:], z_ps[:], b1_sb[:], 0.0,
                            mybir.AluOpType.add, mybir.AluOpType.max,
                        )
                    else:
                        nc.scalar.activation(z_sb[:], z_ps[:], RELU, bias=b1_sb[:])
                    nc.tensor.matmul(
                        y_ps[:],
                        w2_sb[:],
                        z_sb[:],
                        start=(l == 0),
                        stop=(l == L - 1),
                    )
                nc.scalar.activation(out_t[:], y_ps[:], IDENT, bias=b2_sb[:])
                nc.sync.dma_start(out[:, ts(i, M)], out_t[:])

    nc.compile()
    return nc


def prep_host(x, atomic_type, emb, Wt, bt, eps, W1, b1, W2, b2, nc_n=NC_N,
              ncores=NCORES):
    """Host-side prep: fold eps into weights, build per-core input maps."""
    x = np.asarray(x, dtype=np.float32)
    idx = np.asarray(atomic_type).astype(np.int64)
    emb = np.asarray(emb, dtype=np.float32)
    Wt = np.asarray(Wt, dtype=np.float32)
    bt = np.asarray(bt, dtype=np.float32)
    W1 = np.asarray(W1, dtype=np.float32)
    b1 = np.asarray(b1, dtype=np.float32)
    W2 = np.asarray(W2, dtype=np.float32)
    b2 = np.asarray(b2, dtype=np.float32)
    scale = 1.0 + np.float32(np.asarray(eps).reshape(-1)[0])

    # W1 folded through the propagate step (eps-scales cancel in the products):
    #   z1[l] = x[l] @ (scale*W1) + x[l+/-1] @ (Wt[:C] @ W1) + ohsum[l] @ (T @ W1)
    # with T = emb @ Wt[C:] + bt.
    T = (emb @ Wt[C:]) + bt  # [EMB, C]
    xdt = ml_dtypes.bfloat16 if X_BF16 else np.float32
    w1d = np.ascontiguousarray((W1 * scale).astype(xdt))
    w1x = np.ascontiguousarray(
        (Wt[:C].astype(np.float64) @ W1.astype(np.float64)).astype(xdt)
    )
    tw1 = (_OH_SCALE * (T.astype(np.float64) @ W1.astype(np.float64))).astype(
        ml_dtypes.float8_e4m3
    )
    w2s = np.ascontiguousarray(W2)
    b1c = np.ascontiguousarray(b1.reshape(C, 1))
    b2s = np.ascontiguousarray((np.float32(L) * b2).reshape(C, 1))

    arange_emb = np.arange(EMB, dtype=idx.dtype)
    in_maps = []
    for k in range(ncores):
        n0 = k * nc_n
        xs = x[:, n0 : n0 + nc_n, :]  # [L, nc_n, C]
        xtk = np.ascontiguousarray(xs.transpose(2, 0, 1)).astype(xdt)  # [C, L, nc_n]
        ii = idx[:, n0 : n0 + nc_n]  # [L, nc_n]
        ohb = (ii[:, None, :] == arange_emb[None, :, None]).view(np.uint8)
        ohs = np.zeros((L, EMB, nc_n), dtype=np.uint8)
        ohs[:-1] += ohb[1:]
        ohs[1:] += ohb[:-1]
        ohk = _FP8_LUT[ohs.transpose(1, 0, 2)]  # [EMB, L, nc_n] uint8 bits
        ohk = np.ascontiguousarray(ohk).view(ml_dtypes.float8_e4m3)
        in_maps.append(
            {
                "xt": xtk,
                "oh": ohk,
                "w1d": w1d,
                "w1x": w1x,
                "tw1": tw1,
                "w2": w2s,
                "b1": b1c,
                "b2s": b2s,
            }
        )
    return in_maps


_COMPILED = {}


def get_compiled(nc_n=NC_N, num_devices=NCORES):
    key = (nc_n, num_devices)
    if key not in _COMPILED:
        _COMPILED[key] = build_bass(nc_n, num_devices)
    return _COMPILED[key]


def run_on_hw(in_maps, nc=None, trace=False, **kwargs):
    if nc is None:
        nc = get_compiled()
    return bass_utils.run_bass_kernel_spmd(
        nc, in_maps, core_ids=list(range(len(in_maps))), trace=trace, **kwargs
    )


def kernel(**inputs) -> np.ndarray:
    in_maps = prep_host(
        inputs["x"],
        inputs["atomic_type"],
        inputs["emb"],
        inputs["Wt"],
        inputs["bt"],
        inputs["eps"],
        inputs["W1"],
        inputs["b1"],
        inputs["W2"],
        inputs["b2"],
    )
    res = run_on_hw(in_maps)
    out = np.empty((N_FULL, C), dtype=np.float32)
    for k in range(NCORES):
        out[k * NC_N : (k + 1) * NC_N, :] = res.results[k]["out"].T
    return out


if __name__ == "__main__":
    import reference  # only when run manually inside /root/problem

    inputs = {k: np.asarray(v) for k, v in reference.setup_inputs().items()}
    got = kernel(**inputs)
    want = np.asarray(reference.reference(**inputs))
    err = np.abs(got - want).max() / np.abs(want).max()
    print("rel err:", err)


# revision 13
# speedup vs baseline: 1.1676x; 1.1676x over previous
"""Trainium2 Bass kernel for nn_CustomGINConv (gnn_message_passing).

Reference computation (per path n, L=6 layers, C=128 channels):
    h[l]    = x[l] @ Wt[:C] + emb[idx[l]] @ Wt[C:] + bt
    prop[l] = h[l-1] + h[l+1]                (zero-padded along l)
    u[l]    = (1+eps) * x[l] + prop[l]
    out     = sum_l relu(u[l] @ W1 + b1) @ W2 + b2   -> [N, C]

Kernel strategy (shard N across 8 cores, feature-major on-chip layout):
  * eps is folded into weights host-side:
      Wtx' = Wt[:C]/(1+eps),  T' = (emb @ Wt[C:] + bt)/(1+eps),  W1' = (1+eps)*W1
    so   u[l]/(1+eps) = x[l] + x[l-1]@Wtx' + x[l+1]@Wtx' + T'[idx[l-1]] + T'[idx[l+1]]
    and  relu(u@W1 + b1) = relu((u/(1+eps)) @ W1' + b1).
  * The shift-add along l is linear, so it is absorbed into the PSUM
    accumulation of the tensor engine (no vector shift-adds at all).
  * The embedding gather is a one-hot matmul: the host precomputes
    ohsum[l] = onehot(idx[l-1]) + onehot(idx[l+1]) in fp8e4m3 (exact 0/1/2)
    and the PE contracts it against T' (fp8) into the same PSUM banks.
  * fp32 matmuls use the float32r dtype (full-rate fp32 on TRN2 when the
    moving dim is >= 256).
  * Per 512-column tile: 16 prop matmuls -> 1 stacked DVE add (u, in place)
    -> per l: W1' matmul -> ACT relu+bias -> W2 matmul accumulated into one
    PSUM bank -> ACT copy + 6*b2 bias -> DMA out.
"""

import os
import sys

import numpy as np

sys.path.insert(0, "/opt/trn_rl_repo")

import ml_dtypes  # noqa: E402

import concourse.bass as bass  # noqa: E402
import concourse.tile as tile  # noqa: E402
from concourse import bacc, mybir  # noqa: E402
from concourse import bass_utils  # noqa: E402
from concourse.bass import ts  # noqa: E402

L = 6
N_FULL = 65536
C = 128
EMB = 100
NCORES = 8
NC_N = N_FULL // NCORES  # 8192 rows per core
M = 512  # tile width (columns of the feature-major layout)

F32 = mybir.dt.float32
F32R = mybir.dt.float32r
BF16 = mybir.dt.bfloat16
F8 = mybir.dt.float8e4

# x-path precision: float32r (full fp32 accuracy) or bfloat16 (halves the
# dominant HBM payload; ~0.2% relative output error). Toggle before build_bass().
X_BF16 = os.environ.get("KERNEL_X_BF16", "1") == "1"
# which layers run their relu on DVE instead of ACT (load balancing)
RELU_DVE_LS = (1, 3, 5)
RELU = mybir.ActivationFunctionType.Relu
IDENT = mybir.ActivationFunctionType.Identity

# fp8e4m3 bit patterns for {0, 1/16, 2/16}: the one-hot carries a 1/16
# factor (exact powers of two) and tw1 is pre-scaled by 16 so its small
# entries sit in fp8's normal range instead of the subnormals.
_FP8_LUT = np.array([0x00, 0x18, 0x20], dtype=np.uint8)
_OH_SCALE = np.float32(16.0)


def build_bass(nc_n: int = NC_N, num_devices: int = NCORES,
               repeat: int = 1) -> bass.Bass:
    """Build + compile the per-core Bass program (same program on all cores).

    repeat>1 re-runs the whole tile loop (for timing: on-device work scales
    by `repeat` while dispatch overhead stays fixed)."""
    nc = bacc.Bacc(
        "TRN2",
        target_bir_lowering=False,
        debug=False,
        enable_asserts=False,
        num_devices=num_devices,
    )
    XDT = BF16 if X_BF16 else F32R
    xt = nc.dram_tensor("xt", [C, L, nc_n], XDT, kind="ExternalInput").ap()
    oh = nc.dram_tensor("oh", [EMB, L, nc_n], F8, kind="ExternalInput").ap()
    w1d = nc.dram_tensor("w1d", [C, C], XDT, kind="ExternalInput").ap()
    w1x = nc.dram_tensor("w1x", [C, C], XDT, kind="ExternalInput").ap()
    tw1 = nc.dram_tensor("tw1", [EMB, C], F8, kind="ExternalInput").ap()
    w2 = nc.dram_tensor("w2", [C, C], F32R, kind="ExternalInput").ap()
    b1 = nc.dram_tensor("b1", [C, 1], F32, kind="ExternalInput").ap()
    b2s = nc.dram_tensor("b2s", [C, 1], F32, kind="ExternalInput").ap()
    out = nc.dram_tensor("out", [C, nc_n], F32, kind="ExternalOutput").ap()

    nt = nc_n // M
    with tile.TileContext(nc) as tc:
        with (
            tc.tile_pool(name="consts", bufs=1) as consts,
            tc.tile_pool(name="xp", bufs=2) as xp,
            tc.tile_pool(name="ohp", bufs=2) as ohp,
            tc.tile_pool(name="zp", bufs=4) as zp,
            tc.tile_pool(name="outp", bufs=2) as outp,
            tc.tile_pool(name="pp", bufs=1, space="PSUM") as pp,
        ):
            w1d_sb = consts.tile([C, C], XDT, tag="w1d")
            nc.sync.dma_start(w1d_sb[:], w1d)
            w1x_sb = consts.tile([C, C], XDT, tag="w1x")
            nc.sync.dma_start(w1x_sb[:], w1x)
            tw1_sb = consts.tile([EMB, C], F8, tag="tw1")
            nc.sync.dma_start(tw1_sb[:], tw1)
            w2_sb = consts.tile([C, C], F32R, tag="w2")
            nc.sync.dma_start(w2_sb[:], w2)
            b1_sb = consts.tile([C, 1], F32, tag="b1")
            nc.sync.dma_start(b1_sb[:], b1)
            b2_sb = consts.tile([C, 1], F32, tag="b2")
            nc.sync.dma_start(b2_sb[:], b2s)

            for i_rep in range(repeat * nt):
                i = i_rep % nt
                xt_t = xp.tile([C, L, M], XDT, tag="xt")
                nc.sync.dma_start(xt_t[:], xt[:, :, ts(i, M)])
                oh_t = ohp.tile([EMB, L, M], F8, tag="oh")
                nc.sync.dma_start(oh_t[:], oh[:, :, ts(i, M)])

                # xs[l] = x[l-1] + x[l+1] for interior l (one stacked DVE op);
                # boundary layers use the single neighbor directly.
                xs_t = xp.tile([C, L - 2, M], XDT, tag="xs")
                nc.vector.tensor_tensor(
                    xs_t[:], xt_t[:, 0 : L - 2, :], xt_t[:, 2:L, :],
                    mybir.AluOpType.add,
                )

                # z1[l] (pre-relu) accumulates directly in a PSUM bank:
                #   W1'^T x[l] + (Wtx@W1)^T (x[l-1]+x[l+1]) + (T@W1)^T ohsum[l]
                y_ps = pp.tile([C, M], F32, tag="y", bufs=2)
                out_t = outp.tile([C, M], F32, tag="out")
                for l in range(L):
                    z_ps = pp.tile([C, M], F32, tag="z1", bufs=6)
                    nc.tensor.matmul(
                        z_ps[:], w1d_sb[:], xt_t[:, l, :], start=True, stop=False
                    )
                    nbr = (
                        xt_t[:, 1, :] if l == 0
                        else xt_t[:, L - 2, :] if l == L - 1
                        else xs_t[:, l - 1, :]
                    )
                    nc.tensor.matmul(z_ps[:], w1x_sb[:], nbr, start=False, stop=False)
                    nc.tensor.matmul(
                        z_ps[:], tw1_sb[:], oh_t[:, l, :], start=False, stop=True
                    )
                    z_sb = zp.tile([C, M], F32R, tag="z")
                    if l in RELU_DVE_LS:
                        nc.vector.tensor_scalar(
                            z_sb[:], z_ps[:], b1_sb[:], 0.0,
                            mybir.AluOpType.add, mybir.AluOpType.max,
                        )
                    else:
                        nc.scalar.activation(z_sb[:], z_ps[:], RELU, bias=b1_sb[:])
                    nc.tensor.matmul(
                        y_ps[:],
                        w2_sb[:],
                        z_sb[:],
                        start=(l == 0),
                        stop=(l == L - 1),
                    )
                nc.scalar.activation(out_t[:], y_ps[:], IDENT, bias=b2_sb[:])
                nc.sync.dma_start(out[:, ts(i, M)], out_t[:])

    nc.compile()
    return nc


def prep_host(x, atomic_type, emb, Wt, bt, eps, W1, b1, W2, b2, nc_n=NC_N,
              ncores=NCORES):
    """Host-side prep: fold eps into weights, build per-core input maps."""
    x = np.asarray(x, dtype=np.float32)
    idx = np.asarray(atomic_type).astype(np.int64)
    emb = np.asarray(emb, dtype=np.float32)
    Wt = np.asarray(Wt, dtype=np.float32)
    bt = np.asarray(bt, dtype=np.float32)
    W1 = np.asarray(W1, dtype=np.float32)
    b1 = np.asarray(b1, dtype=np.float32)
    W2 = np.asarray(W2, dtype=np.float32)
    b2 = np.asarray(b2, dtype=np.float32)
    scale = 1.0 + np.float32(np.asarray(eps).reshape(-1)[0])

    # W1 folded through the propagate step (eps-scales cancel in the products):
    #   z1[l] = x[l] @ (scale*W1) + x[l+/-1] @ (Wt[:C] @ W1) + ohsum[l] @ (T @ W1)
    # with T = emb @ Wt[C:] + bt.
    T = (emb @ Wt[C:]) + bt  # [EMB, C]
    xdt = ml_dtypes.bfloat16 if X_BF16 else np.float32
    w1d = np.ascontiguousarray((W1 * scale).astype(xdt))
    w1x = np.ascontiguousarray(
        (Wt[:C].astype(np.float64) @ W1.astype(np.float64)).astype(xdt)
    )
    tw1 = (_OH_SCALE * (T.astype(np.float64) @ W1.astype(np.float64))).astype(
        ml_dtypes.float8_e4m3
    )
    w2s = np.ascontiguousarray(W2)
    b1c = np.ascontiguousarray(b1.reshape(C, 1))
    b2s = np.ascontiguousarray((np.float32(L) * b2).reshape(C, 1))

    arange_emb = np.arange(EMB, dtype=idx.dtype)
    in_maps = []
    for k in range(ncores):
        n0 = k * nc_n
        xs = x[:, n0 : n0 + nc_n, :]  # [L, nc_n, C]
        xtk = np.ascontiguousarray(xs.transpose(2, 0, 1)).astype(xdt)  # [C, L, nc_n]
        ii = idx[:, n0 : n0 + nc_n]  # [L, nc_n]
        ohb = (ii[:, None, :] == arange_emb[None, :, None]).view(np.uint8)
        ohs = np.zeros((L, EMB, nc_n), dtype=np.uint8)
        ohs[:-1] += ohb[1:]
        ohs[1:] += ohb[:-1]
        ohk = _FP8_LUT[ohs.transpose(1, 0, 2)]  # [EMB, L, nc_n] uint8 bits
        ohk = np.ascontiguousarray(ohk).view(ml_dtypes.float8_e4m3)
        in_maps.append(
            {
                "xt": xtk,
                "oh": ohk,
                "w1d": w1d,
                "w1x": w1x,
                "tw1": tw1,
                "w2": w2s,
                "b1": b1c,
                "b2s": b2s,
            }
        )
    return in_maps


_COMPILED = {}


def get_compiled(nc_n=NC_N, num_devices=NCORES):
    key = (nc_n, num_devices)
    if key not in _COMPILED:
        _COMPILED[key] = build_bass(nc_n, num_devices)
    return _COMPILED[key]


def run_on_hw(in_maps, nc=None, trace=False, **kwargs):
    if nc is None:
        nc = get_compiled()
    return bass_utils.run_bass_kernel_spmd(
        nc, in_maps, core_ids=list(range(len(in_maps))), trace=trace, **kwargs
    )


def kernel(**inputs) -> np.ndarray:
    in_maps = prep_host(
        inputs["x"],
        inputs["atomic_type"],
        inputs["emb"],
        inputs["Wt"],
        inputs["bt"],
        inputs["eps"],
        inputs["W1"],
        inputs["b1"],
        inputs["W2"],
        inputs["b2"],
    )
    res = run_on_hw(in_maps)
    out = np.empty((N_FULL, C), dtype=np.float32)
    for k in range(NCORES):
        out[k * NC_N : (k + 1) * NC_N, :] = res.results[k]["out"].T
    return out


if __name__ == "__main__":
    import reference  # only when run manually inside /root/problem

    inputs = {k: np.asarray(v) for k, v in reference.setup_inputs().items()}
    got = kernel(**inputs)
    want = np.asarray(reference.reference(**inputs))
    err = np.abs(got - want).max() / np.abs(want).max()
    print("rel err:", err)


# revision 14
# speedup vs baseline: 5.3289x; 4.5640x over previous
"""Trainium2 Bass kernel for nn_CustomGINConv (gnn_message_passing).

Reference computation (per path n, L=6 layers, C=128 channels):
    h[l]    = x[l] @ Wt[:C] + emb[idx[l]] @ Wt[C:] + bt
    prop[l] = h[l-1] + h[l+1]                (zero-padded along l)
    u[l]    = (1+eps) * x[l] + prop[l]
    out     = sum_l relu(u[l] @ W1 + b1) @ W2 + b2   -> [N, C]

Kernel strategy (shard N across 8 cores, feature-major on-chip layout):
  * Everything linear before the relu is folded host-side. With
    T = emb @ Wt[C:] + bt and s = 1+eps (the eps scales cancel):
      z1[l] = x[l] @ (s*W1) + (x[l-1]+x[l+1]) @ (Wt[:C] @ W1)
              + ohsum[l] @ (T @ W1)
    where ohsum[l] = onehot(idx[l-1]) + onehot(idx[l+1]).
    Then out = sum_l relu(z1[l] + b1) @ W2 + L*b2.
  * The propagate shift-add is linear, so it is absorbed into PSUM
    accumulation of the tensor engine plus one stacked DVE add
    (xs[l] = x[l-1]+x[l+1]).
  * The embedding gather is a one-hot matmul: the host precomputes ohsum
    in fp8e4m3 with values {0, 1/16, 2/16} (exact) against a 16x-scaled
    (T @ W1) table so its small entries stay in fp8's normal range.
  * The x path is bf16 (halves the dominant HBM payload; set
    KERNEL_X_BF16=0 for full-fp32 float32r). The z/W2 path is float32r
    (full-precision fp32 matmul at 1 cycle/row on TRN2).
  * Per 512-column tile and layer l: 3 matmuls accumulate z1[l] in its own
    PSUM bank -> relu+bias (alternating ScalarE/VectorE) -> W2 matmul
    accumulated into one y PSUM bank -> bias copy -> DMA out.
"""

import os
import sys

import numpy as np

sys.path.insert(0, "/opt/trn_rl_repo")

import ml_dtypes  # noqa: E402

import concourse.bass as bass  # noqa: E402
import concourse.tile as tile  # noqa: E402
from concourse import bacc, mybir  # noqa: E402
from concourse import bass_utils  # noqa: E402
from concourse.bass import ts  # noqa: E402

L = 6
N_FULL = 65536
C = 128
EMB = 100
NCORES = 8
NC_N = N_FULL // NCORES  # 8192 rows per core
M = 512  # tile width (columns of the feature-major layout)

F32 = mybir.dt.float32
F32R = mybir.dt.float32r
BF16 = mybir.dt.bfloat16
F8 = mybir.dt.float8e4

# x-path precision: float32r (full fp32 accuracy) or bfloat16 (halves the
# dominant HBM payload; ~0.2% relative output error). Toggle before build_bass().
X_BF16 = os.environ.get("KERNEL_X_BF16", "1") == "1"
# which layers run their relu on DVE instead of ACT (load balancing)
RELU_DVE_LS = (1, 3, 5)
RELU = mybir.ActivationFunctionType.Relu
IDENT = mybir.ActivationFunctionType.Identity

# fp8e4m3 bit patterns for {0, 1/16, 2/16}: the one-hot carries a 1/16
# factor (exact powers of two) and tw1 is pre-scaled by 16 so its small
# entries sit in fp8's normal range instead of the subnormals.
_FP8_LUT = np.array([0x00, 0x18, 0x20], dtype=np.uint8)
_OH_SCALE = np.float32(16.0)


def build_bass(nc_n: int = NC_N, num_devices: int = NCORES,
               repeat: int = 1) -> bass.Bass:
    """Build + compile the per-core Bass program (same program on all cores).

    repeat>1 re-runs the whole tile loop (for timing: on-device work scales
    by `repeat` while dispatch overhead stays fixed)."""
    nc = bacc.Bacc(
        "TRN2",
        target_bir_lowering=False,
        debug=False,
        enable_asserts=False,
        num_devices=num_devices,
    )
    XDT = BF16 if X_BF16 else F32R
    xt = nc.dram_tensor("xt", [C, L, nc_n], XDT, kind="ExternalInput").ap()
    oh = nc.dram_tensor("oh", [EMB, L, nc_n], F8, kind="ExternalInput").ap()
    w1d = nc.dram_tensor("w1d", [C, C], XDT, kind="ExternalInput").ap()
    w1x = nc.dram_tensor("w1x", [C, C], XDT, kind="ExternalInput").ap()
    tw1 = nc.dram_tensor("tw1", [EMB, C], F8, kind="ExternalInput").ap()
    w2 = nc.dram_tensor("w2", [C, C], F32R, kind="ExternalInput").ap()
    b1 = nc.dram_tensor("b1", [C, 1], F32, kind="ExternalInput").ap()
    b2s = nc.dram_tensor("b2s", [C, 1], F32, kind="ExternalInput").ap()
    out = nc.dram_tensor("out", [C, nc_n], F32, kind="ExternalOutput").ap()

    nt = nc_n // M
    with tile.TileContext(nc) as tc:
        with (
            tc.tile_pool(name="consts", bufs=1) as consts,
            tc.tile_pool(name="xp", bufs=2) as xp,
            tc.tile_pool(name="ohp", bufs=2) as ohp,
            tc.tile_pool(name="zp", bufs=4) as zp,
            tc.tile_pool(name="outp", bufs=2) as outp,
            tc.tile_pool(name="pp", bufs=1, space="PSUM") as pp,
        ):
            w1d_sb = consts.tile([C, C], XDT, tag="w1d")
            nc.sync.dma_start(w1d_sb[:], w1d)
            w1x_sb = consts.tile([C, C], XDT, tag="w1x")
            nc.sync.dma_start(w1x_sb[:], w1x)
            tw1_sb = consts.tile([EMB, C], F8, tag="tw1")
            nc.sync.dma_start(tw1_sb[:], tw1)
            w2_sb = consts.tile([C, C], F32R, tag="w2")
            nc.sync.dma_start(w2_sb[:], w2)
            b1_sb = consts.tile([C, 1], F32, tag="b1")
            nc.sync.dma_start(b1_sb[:], b1)
            b2_sb = consts.tile([C, 1], F32, tag="b2")
            nc.sync.dma_start(b2_sb[:], b2s)

            for i_rep in range(repeat * nt):
                i = i_rep % nt
                xt_t = xp.tile([C, L, M], XDT, tag="xt")
                nc.sync.dma_start(xt_t[:], xt[:, :, ts(i, M)])
                oh_t = ohp.tile([EMB, L, M], F8, tag="oh")
                nc.sync.dma_start(oh_t[:], oh[:, :, ts(i, M)])

                # xs[l] = x[l-1] + x[l+1] for interior l (one stacked DVE op);
                # boundary layers use the single neighbor directly.
                xs_t = xp.tile([C, L - 2, M], XDT, tag="xs")
                nc.vector.tensor_tensor(
                    xs_t[:], xt_t[:, 0 : L - 2, :], xt_t[:, 2:L, :],
                    mybir.AluOpType.add,
                )

                # z1[l] (pre-relu) accumulates directly in a PSUM bank:
                #   W1'^T x[l] + (Wtx@W1)^T (x[l-1]+x[l+1]) + (T@W1)^T ohsum[l]
                y_ps = pp.tile([C, M], F32, tag="y", bufs=2)
                out_t = outp.tile([C, M], F32, tag="out")
                for l in range(L):
                    z_ps = pp.tile([C, M], F32, tag="z1", bufs=6)
                    nc.tensor.matmul(
                        z_ps[:], w1d_sb[:], xt_t[:, l, :], start=True, stop=False
                    )
                    nbr = (
                        xt_t[:, 1, :] if l == 0
                        else xt_t[:, L - 2, :] if l == L - 1
                        else xs_t[:, l - 1, :]
                    )
                    nc.tensor.matmul(z_ps[:], w1x_sb[:], nbr, start=False, stop=False)
                    nc.tensor.matmul(
                        z_ps[:], tw1_sb[:], oh_t[:, l, :], start=False, stop=True
                    )
                    z_sb = zp.tile([C, M], F32R, tag="z")
                    if l in RELU_DVE_LS:
                        nc.vector.tensor_scalar(
                            z_sb[:], z_ps[:], b1_sb[:], 0.0,
                            mybir.AluOpType.add, mybir.AluOpType.max,
                        )
                    else:
                        nc.scalar.activation(z_sb[:], z_ps[:], RELU, bias=b1_sb[:])
                    nc.tensor.matmul(
                        y_ps[:],
                        w2_sb[:],
                        z_sb[:],
                        start=(l == 0),
                        stop=(l == L - 1),
                    )
                nc.scalar.activation(out_t[:], y_ps[:], IDENT, bias=b2_sb[:])
                nc.sync.dma_start(out[:, ts(i, M)], out_t[:])

    nc.compile()
    return nc


def prep_host(x, atomic_type, emb, Wt, bt, eps, W1, b1, W2, b2, nc_n=NC_N,
              ncores=NCORES):
    """Host-side prep: fold eps into weights, build per-core input maps."""
    x = np.asarray(x, dtype=np.float32)
    idx = np.asarray(atomic_type).astype(np.int64)
    emb = np.asarray(emb, dtype=np.float32)
    Wt = np.asarray(Wt, dtype=np.float32)
    bt = np.asarray(bt, dtype=np.float32)
    W1 = np.asarray(W1, dtype=np.float32)
    b1 = np.asarray(b1, dtype=np.float32)
    W2 = np.asarray(W2, dtype=np.float32)
    b2 = np.asarray(b2, dtype=np.float32)
    scale = 1.0 + np.float32(np.asarray(eps).reshape(-1)[0])

    # W1 folded through the propagate step (eps-scales cancel in the products):
    #   z1[l] = x[l] @ (scale*W1) + x[l+/-1] @ (Wt[:C] @ W1) + ohsum[l] @ (T @ W1)
    # with T = emb @ Wt[C:] + bt.
    T = (emb @ Wt[C:]) + bt  # [EMB, C]
    xdt = ml_dtypes.bfloat16 if X_BF16 else np.float32
    w1d = np.ascontiguousarray((W1 * scale).astype(xdt))
    w1x = np.ascontiguousarray(
        (Wt[:C].astype(np.float64) @ W1.astype(np.float64)).astype(xdt)
    )
    tw1 = (_OH_SCALE * (T.astype(np.float64) @ W1.astype(np.float64))).astype(
        ml_dtypes.float8_e4m3
    )
    w2s = np.ascontiguousarray(W2)
    b1c = np.ascontiguousarray(b1.reshape(C, 1))
    b2s = np.ascontiguousarray((np.float32(L) * b2).reshape(C, 1))

    arange_emb = np.arange(EMB, dtype=idx.dtype)
    in_maps = []
    for k in range(ncores):
        n0 = k * nc_n
        xs = x[:, n0 : n0 + nc_n, :]  # [L, nc_n, C]
        xtk = np.ascontiguousarray(xs.transpose(2, 0, 1)).astype(xdt)  # [C, L, nc_n]
        ii = idx[:, n0 : n0 + nc_n]  # [L, nc_n]
        ohb = (ii[:, None, :] == arange_emb[None, :, None]).view(np.uint8)
        ohs = np.zeros((L, EMB, nc_n), dtype=np.uint8)
        ohs[:-1] += ohb[1:]
        ohs[1:] += ohb[:-1]
        ohk = _FP8_LUT[ohs.transpose(1, 0, 2)]  # [EMB, L, nc_n] uint8 bits
        ohk = np.ascontiguousarray(ohk).view(ml_dtypes.float8_e4m3)
        in_maps.append(
            {
                "xt": xtk,
                "oh": ohk,
                "w1d": w1d,
                "w1x": w1x,
                "tw1": tw1,
                "w2": w2s,
                "b1": b1c,
                "b2s": b2s,
            }
        )
    return in_maps


_COMPILED = {}


def get_compiled(nc_n=NC_N, num_devices=NCORES):
    key = (nc_n, num_devices)
    if key not in _COMPILED:
        _COMPILED[key] = build_bass(nc_n, num_devices)
    return _COMPILED[key]


def run_on_hw(in_maps, nc=None, trace=False, **kwargs):
    if nc is None:
        nc = get_compiled()
    return bass_utils.run_bass_kernel_spmd(
        nc, in_maps, core_ids=list(range(len(in_maps))), trace=trace, **kwargs
    )


def kernel(**inputs) -> np.ndarray:
    in_maps = prep_host(
        inputs["x"],
        inputs["atomic_type"],
        inputs["emb"],
        inputs["Wt"],
        inputs["bt"],
        inputs["eps"],
        inputs["W1"],
        inputs["b1"],
        inputs["W2"],
        inputs["b2"],
    )
    res = run_on_hw(in_maps)
    out = np.empty((N_FULL, C), dtype=np.float32)
    for k in range(NCORES):
        out[k * NC_N : (k + 1) * NC_N, :] = res.results[k]["out"].T
    return out


if __name__ == "__main__":
    import reference  # only when run manually inside /root/problem

    inputs = {k: np.asarray(v) for k, v in reference.setup_inputs().items()}
    got = kernel(**inputs)
    want = np.asarray(reference.reference(**inputs))
    err = np.abs(got - want).max() / np.abs(want).max()
    print("rel err:", err)
